# revision 22
# baseline (speedup 1.0000x reference)
"""APPNP (K=2, alpha=0.5) GNN propagation + linear head on 8 TRN2 NeuronCores.

Sharding: dst nodes across cores (12500 rows each). Per hop, per core:
edge-source rows are fetched from the full bf16 feature table in HBM with
dma_gather (int16 source banks, 4 SWDGE queues); the TensorEngine
accumulates S^T @ gathered_rows into the dst-block PSUM tile, where
S [128 x 128] bf16 carries the GCN edge norms (host-built, streamed).
Self-loops are applied as a diagonal term on the VectorEngine. Between
hops an AllGather rebuilds the full table. After hop 2 each block is
relu'd and pushed through W/bias on the TensorEngine (PE transpose + two
matmuls), f32 out.

Edges are bucketed per (dst-block, src-bank) into fixed-size gather calls
with trailing -1 index padding so the SPMD graph is identical on every
core; per-call valid counts are runtime data (reg_load from SBUF).
"""

import sys
import types

import numpy as np
import ml_dtypes

BF16 = ml_dtypes.bfloat16
FP8 = ml_dtypes.float8_e4m3

N_NODES = 100000
D = 256
ALPHA = 0.5
NCORES = 8
RPC = N_NODES // NCORES                      # rows per core: 12500
NBLOCKS = (RPC + 127) // 128                 # 98 (last block has 84 rows)
LASTB = RPC - 128 * (NBLOCKS - 1)            # 84
BANKS = [0, 25000, 50000, 75000, 100000]
NBANKS = 4
QROWS = 3125                                 # rows per core per AG quarter
NT = [9, 9, 9, 9]                            # tiles per (block, bank) call
NT_TOT = sum(NT)                             # 36
CALL_OFF = [0, 9, 18, 27]
NIDX = [t * 128 for t in NT]
IDXW = [n // 16 for n in NIDX]
IDXW_TOT = sum(IDXW)
NUNITS = NBLOCKS * NBANKS
NSTG = 12


def _install_ntff_hook_shim():
    if "antenv.axon_hooks" in sys.modules:
        return
    try:
        from trn_agent_boot.trn_boot import _ntff_profile_via_ctypes

        hook = _ntff_profile_via_ctypes("/opt/axon/libaxon_pjrt.so")
    except Exception:
        return
    mod = types.ModuleType("antenv.axon_hooks")
    mod._hook = hook
    mod.get_axon_ntff_profile_hook = lambda: mod._hook
    mod.set_axon_ntff_profile_hook = lambda h: setattr(mod, "_hook", h)
    sys.modules["antenv.axon_hooks"] = mod
    import antenv

    antenv.axon_hooks = mod


# ======================================================================
# Host-side preprocessing
# ======================================================================

def _gcn_norm_host(edge_index, edge_weight, n):
    src = edge_index[0].astype(np.int64)
    dst = edge_index[1].astype(np.int64)
    w = edge_weight.astype(np.float64)
    deg = np.zeros(n, dtype=np.float64)
    np.add.at(deg, dst, w)
    deg += 1.0
    dinv = 1.0 / np.sqrt(np.maximum(deg, 1e-12))
    norm = (dinv[src] * w * dinv[dst]).astype(np.float32)
    return src, dst, norm, dinv.astype(np.float32)


def _rowof(node):
    c = node // RPC
    r = node % RPC
    return (r // QROWS) * (NCORES * QROWS) + c * QROWS + (r % QROWS)


def _pack_core(core, src, dst, norm):
    lo = core * RPC
    m = (dst >= lo) & (dst < lo + RPC)
    es, ed, en = _rowof(src[m]), dst[m] - lo, norm[m]

    blk = ed >> 7
    bank = es // (NCORES * QROWS)
    unit = blk * NBANKS + bank
    order = np.lexsort((ed, unit))
    es, ed, en, unit = es[order], ed[order], en[order], unit[order]
    ub = np.searchsorted(unit, np.arange(NUNITS + 1))

    idx = np.full((128, NBLOCKS * IDXW_TOT), -1, dtype=np.int16)
    cnt = np.zeros((1, NUNITS), dtype=np.int32)
    smat = np.zeros((NBLOCKS, 128, NT_TOT * 128), dtype=FP8)

    for u in range(NUNITS):
        a, b = ub[u], ub[u + 1]
        c = b - a
        bb, k = divmod(u, NBANKS)
        if c > NIDX[k]:
            raise RuntimeError(f"unit {u} overflow {c} > {NIDX[k]}")
        cnt[0, u] = c
        if c == 0:
            continue
        flat = np.full(NIDX[k], -1, dtype=np.int16)
        flat[:c] = (es[a:b] - BANKS[k]).astype(np.int16)
        wrapped = flat.reshape(IDXW[k], 16).T
        col0 = bb * IDXW_TOT + sum(IDXW[:k])
        idx[:, col0 : col0 + IDXW[k]] = np.tile(wrapped, (8, 1))
        t = CALL_OFF[k] + np.arange(c) // 128
        p = np.arange(c) % 128
        dloc = (ed[a:b] & 127).astype(np.int64)
        smat[bb, p, t * 128 + dloc] = en[a:b].astype(FP8)
    return idx, cnt, smat


def _preprocess(x, edge_index, edge_weight, W, b):
    src, dst, norm, dinv = _gcn_norm_host(edge_index, edge_weight, N_NODES)
    x32 = np.asarray(x, dtype=np.float32)
    rowof = _rowof(np.arange(N_NODES))
    x_rm = np.empty_like(x32)
    x_rm[rowof] = x32
    x_bf = np.ascontiguousarray(x_rm).astype(FP8)
    Wt = np.ascontiguousarray(np.asarray(W, dtype=np.float32).T).astype(BF16)
    bias = np.ascontiguousarray(np.tile(np.asarray(b, dtype=np.float32).reshape(1, D), (128, 1)))
    ident = np.eye(128, dtype=np.float32).astype(BF16)

    per_core = []
    for c in range(NCORES):
        idx, cnt, smat = _pack_core(c, src, dst, norm)
        sl = slice(c * RPC, (c + 1) * RPC)
        xh = np.ascontiguousarray((ALPHA * x32[sl]).astype(BF16))
        hd0 = np.ascontiguousarray(x32[sl].astype(BF16))
        dsq = np.ascontiguousarray(
            ((1.0 - ALPHA) * dinv[sl] * dinv[sl]).astype(np.float32).reshape(RPC, 1)
        )
        per_core.append(dict(idx=idx, cnt=cnt, smat=smat, xh=xh, dsq=dsq, hd0=hd0))
    return per_core, x_bf, Wt, bias, ident


# ======================================================================
# Device kernel
# ======================================================================

def _build():
    import concourse.bacc as bacc
    import concourse.mybir as mybir
    from concourse.library_config import mlp

    nc = bacc.Bacc("TRN2", num_swdge_queues=4)
    dt = mybir.dt
    Alu = mybir.AluOpType

    htab = nc.declare_dram_parameter("htab", [N_NODES, D], dt.float8e4, isOutput=False)
    idx_e = nc.declare_dram_parameter("idx", [128, NBLOCKS * IDXW_TOT], dt.int16, isOutput=False)
    cnt_e = nc.declare_dram_parameter("cnt", [1, NUNITS], dt.int32, isOutput=False)
    s_e = nc.declare_dram_parameter("smat", [NBLOCKS, 128, NT_TOT * 128], dt.float8e4, isOutput=False)
    xh_e = nc.declare_dram_parameter("xh", [RPC, D], dt.bfloat16, isOutput=False)
    dsq_e = nc.declare_dram_parameter("dsq", [RPC, 1], dt.float32, isOutput=False)
    wt_e = nc.declare_dram_parameter("wt", [D, D], dt.bfloat16, isOutput=False)
    b_e = nc.declare_dram_parameter("bias", [128, D], dt.float32, isOutput=False)
    id_e = nc.declare_dram_parameter("ident", [128, 128], dt.bfloat16, isOutput=False)
    hd0_e = nc.declare_dram_parameter("hd0", [RPC, D], dt.bfloat16, isOutput=False)
    out_e = nc.declare_dram_parameter("out", [RPC, D], dt.float32, isOutput=True)

    h1loc = nc.dram_tensor("h1loc", [RPC, D], dt.float8e4)
    h1full = nc.dram_tensor("h1full", [N_NODES, D], dt.float8e4, addr_space="Shared")

    # gather schedule: queue = (blk+k)%4, buffer = 3*q + (per-queue seq % 3).
    # hop-1 prologue reorders issue so banks 0-2 of blocks 0-2 run under the
    # tail AllGather; bank-3 calls (needing quarter 3) come after.
    def _issue_order(h):
        if h == 0:
            return [(blk, k) for blk in range(NBLOCKS) for k in range(NBANKS)]
        pro = [(b, k) for b in range(3) for k in range(3)] + [(b, 3) for b in range(3)]
        return pro + [(blk, k) for blk in range(3, NBLOCKS) for k in range(NBANKS)]

    g_sched = {}   # (h,u) -> (q, sb, sem_target, prev_gcall_on_buffer or None)
    _qn = [0] * 4
    _g = [0] * NSTG
    _buf_hist = {i: [] for i in range(NSTG)}
    for h in range(2):
        for blk, k in _issue_order(h):
            u = blk * NBANKS + k
            q = (blk + k) % 4
            sbuf = 3 * q + (_qn[q] % 3)
            _qn[q] += 1
            _g[sbuf] += 16
            prev = _buf_hist[sbuf][-1] if _buf_hist[sbuf] else None
            _buf_hist[sbuf].append(h * NUNITS + u)
            g_sched[(h, u)] = (q, sbuf, _g[sbuf], prev)

    from contextlib import ExitStack

    with ExitStack() as _st:
        block = _st.enter_context(nc.Block())
        sem = lambda n: _st.enter_context(nc.semaphore(n))
        sb = lambda n, shp, dty: _st.enter_context(nc.sbuf_tensor(n, shp, dty))
        ps_ = lambda n, shp, dty: _st.enter_context(nc.psum_tensor(n, shp, dty))
        ini = sem("ini"); ini2 = sem("ini2"); mz = sem("mz"); ds = sem("ds"); cc = sem("cc")
        s_ld = [sem("s_ld0"), sem("s_ld1"), sem("s_ld2")]; x_ld = [sem("x_ld0"), sem("x_ld1")]
        wr = [sem("wr0"), sem("wr1")]
        pe_b = sem("pe_b"); cmb = sem("cmb"); tr = sem("tr")
        h2t = sem("h2t"); fm = sem("fm"); ob = sem("ob")
        gs = [sem(f"g{i}") for i in range(NSTG)]
        idx_sb = sb("idx_sb", [128, NBLOCKS * IDXW_TOT], dt.int16)
        cnt_sb = sb("cnt_sb", [1, NUNITS], dt.int32)
        stg = sb("stg", [128, NSTG, 12, D], dt.float8e4)
        s_sb = sb("s_sb", [128, 3, NT_TOT, 128], dt.float8e4)
        xh_sb = sb("xh_sb", [128, 2, D], dt.bfloat16)
        hd_sb = sb("hd_sb", [128, 2, D], dt.bfloat16)
        hd8_sb = sb("hd8_sb", [128, 2, D], dt.float8e4)
        hn8_sb = sb("hn8_sb", [128, 2, D], dt.float8e4)
        dsq_sb = sb("dsq_sb", [128, 2, 1], dt.float32)
        t1_sb = sb("t1_sb", [128, 2, D], dt.float32)
        t2_sb = sb("t2_sb", [128, 2, D], dt.float32)
        hn_sb = sb("hn_sb", [128, 2, D], dt.bfloat16)
        wt_sb = sb("wt_sb", [128, 2, D], dt.bfloat16)
        b_sb = sb("b_sb", [128, D], dt.float32)
        id_sb = sb("id_sb", [128, 128], dt.bfloat16)
        h2rt_sb = sb("h2rt_sb", [128, 2, 2, 128], dt.bfloat16)
        o_sb = sb("o_sb", [128, 2, D], dt.float32)
        msum = [ps_("msum0", [128, D], dt.float32), ps_("msum1", [128, D], dt.float32)]
        tpsum = [ps_("tpsum0", [128, 2, 128], dt.bfloat16), ps_("tpsum1", [128, 2, 128], dt.bfloat16)]
        opsum = [ps_("opsum0", [128, D], dt.float32), ps_("opsum1", [128, D], dt.float32)]



        @block.sync
        def _(sync):
            sync.dma_start(out=idx_sb[:], in_=idx_e[:]).then_inc(ini, 16)
            sync.dma_start(out=cnt_sb[:], in_=cnt_e[:]).then_inc(ini, 16)
            sync.dma_start(out=wt_sb[:, 0, :], in_=wt_e[0:128, :]).then_inc(ini2, 16)
            sync.dma_start(out=wt_sb[:, 1, :], in_=wt_e[128:256, :]).then_inc(ini2, 16)
            sync.dma_start(out=b_sb[:], in_=b_e[:]).then_inc(ini2, 16)
            sync.dma_start(out=id_sb[:], in_=id_e[:]).then_inc(ini2, 16)

            for h in range(2):
                hd_src = htab if h == 0 else h1loc
                hd_off = RPC * 0  # hd rows are this core's own rows
                if h == 1:
                    sync.wait_ge(wr[0], 16 * ((NBLOCKS - 1) // 2 + 1))
                    sync.wait_ge(wr[1], 16 * (NBLOCKS // 2))
                for blk in range(NBLOCKS):
                    half = blk % 2
                    gblk = h * NBLOCKS + blk
                    rows = 128 if blk < NBLOCKS - 1 else LASTB
                    r0 = blk * 128
                    hs = gblk % 3
                    if gblk >= 3:
                        sync.wait_ge(pe_b, gblk - 2)
                    sync.dma_start(out=s_sb[:, hs, :, :], in_=s_e[blk]).then_inc(
                        s_ld[hs], 16
                    )
                    if gblk >= 2:
                        sync.wait_ge(cmb, gblk - 1)
                    sync.dma_start(
                        out=xh_sb[:rows, half, :], in_=xh_e[r0 : r0 + rows, :]
                    ).then_inc(x_ld[half], 16)
                    sync.dma_start(
                        out=dsq_sb[:rows, half, :], in_=dsq_e[r0 : r0 + rows, :]
                    ).then_inc(x_ld[half], 16)
                    if h == 0:
                        sync.dma_start(
                            out=hd_sb[:rows, half, :],
                            in_=hd0_e[r0 : r0 + rows, :],
                        ).then_inc(x_ld[half], 16)
                    else:
                        sync.dma_start(
                            out=hd8_sb[:rows, half, :],
                            in_=h1loc[r0 : r0 + rows, :],
                        ).then_inc(x_ld[half], 16)
                    if h == 0:
                        sync.wait_ge(cmb, blk + 1)
                        sync.dma_start(
                            out=h1loc[r0 : r0 + rows, :], in_=hn8_sb[:rows, half, :]
                        ).then_inc(wr[half], 16)
                    else:
                        if blk >= 2:
                            bb = blk - 2
                            sync.wait_ge(ob, bb + 1)
                            sync.dma_start(
                                out=out_e[bb * 128 : bb * 128 + 128, :],
                                in_=o_sb[:, bb % 2, :],
                            ).then_inc(wr[bb % 2], 16)
                if h == 1:
                    for bb in (NBLOCKS - 2, NBLOCKS - 1):
                        rows = 128 if bb < NBLOCKS - 1 else LASTB
                        sync.wait_ge(ob, bb + 1)
                        sync.dma_start(
                            out=out_e[bb * 128 : bb * 128 + rows, :],
                            in_=o_sb[:rows, bb % 2, :],
                        ).then_inc(wr[bb % 2], 16)

        @block.gpsimd
        def _(gpsimd):
            gpsimd.load_library(mlp)
            gpsimd.wait_ge(ini, 32)
            gpsimd.wait_ge(mz, 1)
            regs = [gpsimd.alloc_register(f"cntreg{i}") for i in range(12)]
            bregs = lambda gb: regs[(gb % 3) * 4 : (gb % 3) * 4 + 4]

            def gcall(h, blk, k, table):
                u = blk * NBANKS + k
                q, sbuf, _, prev = g_sched[(h, u)]
                col0 = blk * IDXW_TOT + sum(IDXW[:k])
                gpsimd.dma_gather(
                    stg[:, sbuf, 0 : NT[k], :],
                    table[BANKS[k] : BANKS[k + 1], :],
                    idx_sb[:, col0 : col0 + IDXW[k]],
                    NIDX[k],
                    bregs(h * NBLOCKS + blk)[k],
                    D,
                    single_packet=False,
                    queue_num=q,
                ).then_inc(gs[sbuf], 16)

            gpsimd.reg_load(bregs(0), cnt_sb[0:1, 0:4])
            for h in range(2):
                table = htab if h == 0 else h1full
                if h == 1:
                    B = NBLOCKS - 1
                    gpsimd.wait_ge(wr[0], 16 * (B // 2 + 1))
                    gpsimd.wait_ge(wr[1], 16 * ((B + 1) // 2))
                    gpsimd.collective_compute(
                        "AllGather",
                        Alu.bypass,
                        replica_groups=[list(range(NCORES))],
                        ins=[h1loc[3 * QROWS : 4 * QROWS, :]],
                        outs=[h1full[3 * NCORES * QROWS : 4 * NCORES * QROWS, :]],
                    ).then_inc(cc, 1)
                    gpsimd.reg_load(bregs(NBLOCKS + 1), cnt_sb[0:1, 4:8])
                    gpsimd.reg_load(bregs(NBLOCKS + 2), cnt_sb[0:1, 8:12])
                    gpsimd.wait_ge(cc, 3)
                    first_b3 = [True]
                    for blk, k in _issue_order(1)[:12]:
                        if k == 3 and first_b3[0]:
                            gpsimd.wait_ge(cc, 4)
                            first_b3[0] = False
                        prev = g_sched[(1, blk * NBANKS + k)][3]
                        if prev is not None:
                            gpsimd.wait_ge(pe_b, prev // NBANKS + 1)
                        gcall(1, blk, k, table)
                    gpsimd.reg_load(bregs(NBLOCKS + 3), cnt_sb[0:1, 12:16])
                ag_at = {31: (0, 24), 55: (1, 48), 80: (2, 73)}
                for blk in range(3 if h == 1 else 0, NBLOCKS):
                    if h == 0 and blk in ag_at:
                        qq, B = ag_at[blk]
                        gpsimd.wait_ge(wr[0], 16 * (B // 2 + 1))
                        gpsimd.wait_ge(wr[1], 16 * ((B + 1) // 2))
                        gpsimd.collective_compute(
                            "AllGather",
                            Alu.bypass,
                            replica_groups=[list(range(NCORES))],
                            ins=[h1loc[qq * QROWS : (qq + 1) * QROWS, :]],
                            outs=[h1full[qq * NCORES * QROWS : (qq + 1) * NCORES * QROWS, :]],
                        ).then_inc(cc, 1)
                    gblk = h * NBLOCKS + blk
                    _prevs = [
                        g_sched[(h, blk * NBANKS + k)][3] for k in range(NBANKS)
                    ]
                    _pwait = max(
                        (p // NBANKS for p in _prevs if p is not None), default=None
                    )
                    if _pwait is not None:
                        gpsimd.wait_ge(pe_b, _pwait + 1)
                    for k in range(NBANKS):
                        gcall(h, blk, k, table)
                    if gblk < 2 * NBLOCKS - 1:
                        nu = ((gblk + 1) % NBLOCKS) * NBANKS
                        gpsimd.reg_load(bregs(gblk + 1), cnt_sb[0:1, nu : nu + 4])

        @block.tensor
        def _(tensor):
            tensor.wait_ge(ini2, 64)
            for h in range(2):
                for blk in range(NBLOCKS):
                    half = blk % 2
                    gblk = h * NBLOCKS + blk
                    if gblk >= 2:
                        tensor.wait_ge(cmb, gblk - 1)
                    hs = gblk % 3
                    tensor.wait_ge(s_ld[hs], 16 * (gblk // 3 + 1))
                    first = True
                    inst = None
                    for k in range(NBANKS):
                        u = blk * NBANKS + k
                        _, sbuf, gt, _p = g_sched[(h, u)]
                        tensor.wait_ge(gs[sbuf], gt)
                        for t in range(NT[k]):
                            inst = tensor.matmul(
                                msum[half][:],
                                s_sb[:, hs, CALL_OFF[k] + t, :],
                                stg[:, sbuf, t, :],
                                start=first,
                                stop=(k == NBANKS - 1 and t == NT[k] - 1),
                            )
                            first = False
                    inst.then_inc(pe_b, 1)
                    if h == 1:
                        if blk >= 1:
                            bb = blk - 1
                            tensor.wait_ge(cmb, NBLOCKS + bb + 1)
                            if bb >= 2:
                                tensor.wait_ge(h2t, bb - 1)  # tpsum half reuse
                            ti = None
                            for j in range(2):
                                ti = tensor.transpose(
                                    out=tpsum[bb % 2][:, j, :],
                                    in_=hn_sb[:, bb % 2, j * 128 : (j + 1) * 128],
                                    identity=id_sb[:],
                                )
                            ti.then_inc(tr, 1)
                        if blk >= 2:
                            bb = blk - 2
                            tensor.wait_ge(h2t, bb + 1)
                            if bb >= 2:
                                tensor.wait_ge(ob, bb - 1)  # opsum half reuse
                            fi = None
                            for j in range(2):
                                fi = tensor.matmul(
                                    opsum[bb % 2][:],
                                    h2rt_sb[:, bb % 2, j, :],
                                    wt_sb[:, j, :],
                                    start=(j == 0),
                                    stop=(j == 1),
                                )
                            fi.then_inc(fm, 1)
            # epilogue: transposes for last block, final MMs for last two
            bb = NBLOCKS - 1
            tensor.wait_ge(cmb, 2 * NBLOCKS)
            tensor.wait_ge(h2t, bb - 1)
            ti = None
            for j in range(2):
                ti = tensor.transpose(
                    out=tpsum[bb % 2][:, j, :],
                    in_=hn_sb[:, bb % 2, j * 128 : (j + 1) * 128],
                    identity=id_sb[:],
                )
            ti.then_inc(tr, 1)
            for bb in (NBLOCKS - 2, NBLOCKS - 1):
                tensor.wait_ge(h2t, bb + 1)
                tensor.wait_ge(ob, bb - 1)
                fi = None
                for j in range(2):
                    fi = tensor.matmul(
                        opsum[bb % 2][:],
                        h2rt_sb[:, bb % 2, j, :],
                        wt_sb[:, j, :],
                        start=(j == 0),
                        stop=(j == 1),
                    )
                fi.then_inc(fm, 1)

        @block.vector
        def _(vector):
            dsn = [0]
            vector.memset(stg[:], 0.0).then_inc(mz, 1)
            vector.wait_ge(ini2, 64)
            for h in range(2):
                for blk in range(NBLOCKS):
                    half = blk % 2
                    gblk = h * NBLOCKS + blk
                    rows = 128 if blk < NBLOCKS - 1 else LASTB
                    vector.wait_ge(pe_b, gblk + 1)
                    vector.wait_ge(x_ld[half], 48 * (gblk // 2 + 1))
                    if h == 0 and blk >= 2:
                        vector.wait_ge(wr[half], 16 * (blk // 2))
                    if h == 1 and blk <= 1:
                        vector.wait_ge(wr[0], 16 * ((NBLOCKS + 1) // 2))
                        vector.wait_ge(wr[1], 16 * (NBLOCKS // 2))
                    if h == 1 and blk >= 2:
                        vector.wait_ge(tr, blk - 1)
                    # t2 = dsq*hd + xh ; then hn = 0.5*msum + t2 (+relu on hop 1)
                    vector.scalar_tensor_tensor(
                        out=t2_sb[:rows, half, :],
                        in0=(hd_sb if h == 0 else hd8_sb)[:rows, half, :],
                        scalar=dsq_sb[:rows, half, :], in1=xh_sb[:rows, half, :],
                        op0=Alu.mult, op1=Alu.add,
                    ).then_inc(ds, 1)
                    dsn[0] += 1
                    vector.wait_ge(ds, dsn[0])
                    if h == 0:
                        vector.scalar_tensor_tensor(
                            out=hn8_sb[:rows, half, :], in0=msum[half][:rows, :],
                            scalar=0.5, in1=t2_sb[:rows, half, :],
                            op0=Alu.mult, op1=Alu.add,
                        ).then_inc(cmb, 1)
                    else:
                        vector.scalar_tensor_tensor(
                            out=t1_sb[:rows, half, :], in0=msum[half][:rows, :],
                            scalar=0.5, in1=t2_sb[:rows, half, :],
                            op0=Alu.mult, op1=Alu.add,
                        ).then_inc(ds, 1)
                        dsn[0] += 1
                        vector.wait_ge(ds, dsn[0])
                        vector.tensor_scalar(
                            out=hn_sb[:rows, half, :], in0=t1_sb[:rows, half, :],
                            scalar1=0.0, scalar2=None, op0=Alu.max,
                        ).then_inc(cmb, 1)
                    if h == 1:
                        if blk >= 1:
                            bb = blk - 1
                            vector.wait_ge(tr, bb + 1)
                            if bb >= 2:
                                vector.wait_ge(fm, bb - 1)  # h2rt half reuse
                            ci = None
                            for j in range(2):
                                ci = vector.tensor_copy(
                                    out=h2rt_sb[:, bb % 2, j, :],
                                    in_=tpsum[bb % 2][:, j, :],
                                )
                            ci.then_inc(h2t, 1)
                        if blk >= 2:
                            bb = blk - 2
                            vector.wait_ge(fm, bb + 1)
                            if bb >= 2:
                                vector.wait_ge(
                                    wr[bb % 2],
                                    16 * ((NBLOCKS + 1 - bb % 2) // 2) + 16 * ((bb - 2) // 2 + 1),
                                )
                            vector.tensor_tensor(
                                out=o_sb[:, bb % 2, :], in0=opsum[bb % 2][:],
                                in1=b_sb[:],
                                op=Alu.add,
                            ).then_inc(ob, 1)
            # epilogue
            bb = NBLOCKS - 1
            vector.wait_ge(tr, bb + 1)
            vector.wait_ge(fm, bb - 1)
            ci = None
            for j in range(2):
                ci = vector.tensor_copy(
                    out=h2rt_sb[:, bb % 2, j, :], in_=tpsum[bb % 2][:, j, :]
                )
            ci.then_inc(h2t, 1)
            for bb in (NBLOCKS - 2, NBLOCKS - 1):
                vector.wait_ge(fm, bb + 1)
                vector.wait_ge(
                    wr[bb % 2],
                    16 * ((NBLOCKS + 1 - bb % 2) // 2) + 16 * ((bb - 2) // 2 + 1),
                )
                vector.tensor_tensor(
                    out=o_sb[:, bb % 2, :], in0=opsum[bb % 2][:],
                    in1=b_sb[:],
                    op=Alu.add,
                ).then_inc(ob, 1)

    nc.compile()
    return nc


_NC_CACHE = None


def kernel(x, edge_index, edge_weight, W, b):
    _install_ntff_hook_shim()
    global _NC_CACHE
    per_core, x_bf, Wt, bias, ident = _preprocess(x, edge_index, edge_weight, W, b)
    if _NC_CACHE is None:
        _NC_CACHE = _build()
    nc = _NC_CACHE
    in_maps = []
    for c in range(NCORES):
        pc = per_core[c]
        in_maps.append(
            dict(
                htab=x_bf, idx=pc["idx"], cnt=pc["cnt"], smat=pc["smat"],
                xh=pc["xh"], dsq=pc["dsq"], wt=Wt, bias=bias, ident=ident,
                hd0=pc["hd0"],
            )
        )
    from concourse.bass_utils import run_bass_kernel_spmd

    res = run_bass_kernel_spmd(nc, in_maps, list(range(NCORES)))
    out = np.concatenate(
        [np.asarray(res.results[c]["out"]) for c in range(NCORES)], axis=0
    )
    return out.astype(np.float32)

